# revision 25
# baseline (speedup 1.0000x reference)
"""Causal attention (B=4, L=2048, d_model=1024, d_k=d_v=128) on 8 TRN2 NeuronCores.

Key-parity split; v5 snapshot (46430 ns): serialized sync input DMAs,
merged tiles, V projected directly in [key, v] layout, scalar-engine
proj copies, additive f32 causal mask pre-exp on DVE, out DMAs on scalar.
"""

import sys

sys.path.insert(0, "/opt/trn_rl_repo")
sys.path.insert(0, "/opt/trn_rl_repo/concourse")

import ml_dtypes
import numpy as np

import concourse.bass as bass  # noqa: F401
import concourse.mybir as mybir
import concourse.tile as tile
from concourse import bacc
from concourse.bass_utils import run_bass_kernel_spmd

B, L, DM, DK, DV = 4, 2048, 1024, 128, 128
NCH = DM // 128
SLOTS = 8
QB = 16
SCALE = float(DK) ** -0.5
MASKVAL = -1e9

F32 = mybir.dt.float32
BF16 = mybir.dt.bfloat16

PIECES = [
    (0,    512, 0, 2),
    (0,    512, 2, 2),
    (0,    512, 4, 4),
    (512,  512, 0, 8),
    (1024, 512, 0, 8),
    (1536, 512, 0, 8),
]
XIN_COLS = sum(w * cw for _, w, _, cw in PIECES)  # 16384


def build_nc():
    nc = bacc.Bacc()

    xin_ext = nc.declare_dram_parameter("xin", [128, XIN_COLS], BF16, isOutput=False)
    # wq (8 chunks x 128) followed by x piece 0 (2 chunks x 512) per partition
    head_ext = nc.declare_dram_parameter("head", [128, DM + 2 * 512], BF16,
                                         isOutput=False)
    wkv_ext = nc.declare_dram_parameter("wkv", [128, 2 * DM], BF16, isOutput=False)
    mask_ext = nc.declare_dram_parameter("maskT", [128, 256], F32, isOutput=False)
    out_ext = nc.declare_dram_parameter("out", [128, QB * 129], F32, isOutput=True)

    with tile.TileContext(nc) as tc:
        with (
            tc.tile_pool(name="persist", bufs=1) as persist,
            tc.tile_pool(name="mm_ps", bufs=6, space="PSUM") as mm_ps,
            tc.tile_pool(name="z_ps", bufs=2, space="PSUM") as z_ps,
        ):
            head_sb = persist.tile([128, DM + 2 * 512], BF16, tag="head",
                                   name="head")
            nc.sync.dma_start(out=head_sb[:], in_=head_ext[:])
            wq_sb = head_sb[:, 0:DM].rearrange("p (c d) -> p c d", d=128)

            xp = [head_sb[:, DM:DM + 2 * 512].rearrange("p (c w) -> p c w", w=512)]

            def load_piece(j):
                _, w, _, cw = PIECES[j]
                off = sum(wi * cwi for _, wi, _, cwi in PIECES[:j])
                t = persist.tile([128, cw, w], BF16, tag=f"xp{j}", name=f"xp{j}")
                nc.sync.dma_start(
                    out=t[:],
                    in_=xin_ext[:, off:off + cw * w].rearrange(
                        "p (c w) -> p c w", w=w
                    ),
                )
                return t

            xp.append(load_piece(1))
            wkv_sb = persist.tile([128, 2, NCH, 128], BF16, tag="wkv", name="wkv")
            nc.sync.dma_start(
                out=wkv_sb[:],
                in_=wkv_ext.rearrange("p (i c d) -> p i c d", i=2, d=128))
            xp.append(load_piece(2))
            mask_sb = persist.tile([128, 256], F32, tag="mask")
            nc.scalar.dma_start(out=mask_sb[:], in_=mask_ext[:])
            for j in range(3, len(PIECES)):
                xp.append(load_piece(j))

            w_sb = {"wq": wq_sb, "wk": wkv_sb[:, 0], "wv": wkv_sb[:, 1]}

            qt = persist.tile([128, 4 * 512], BF16, tag="qt", name="qt")
            kt = persist.tile([128, 2 * 512], BF16, tag="kt", name="kt")
            va = persist.tile([128, SLOTS * (DV + 1)], BF16, tag="va", name="va")
            nc.vector.memset(va[:], 1.0)
            at = [persist.tile([128, SLOTS * 512], BF16, tag=f"atg{g}",
                               name=f"atg{g}") for g in range(4)]
            z_sb = persist.tile([128, QB * 129], F32, tag="zsb", name="zsb")

            GRP = {"A": [(0, 0, 2), (1, 2, 2), (2, 4, 4)], "B": [(3, 0, 8)],
                   "C": [(4, 0, 8)], "D": [(5, 0, 8)]}

            def warmfill(n):
                # junk matmuls, result never read: keep the PE clock hot
                # while DMA fills (PE is in-order; these slot into stalls)
                wt = mm_ps.tile([128, 256], F32, tag="mm", name="warm")
                for wi in range(n):
                    nc.tensor.matmul(
                        wt[:], va[:, 0:128], va[:, 0:256],
                        start=(wi == 0), stop=(wi == n - 1),
                    )

            warmfill(13)

            def proj_q_A():
                # Q over column group A with warm fillers at the piece
                # boundaries where the DMA stream is still catching up
                w = w_sb["wq"]
                ps = mm_ps.tile([128, 512], F32, tag="mm", name="pwqA")
                for j, clo, cw in GRP["A"]:
                    for cl in range(cw):
                        c = clo + cl
                        nc.tensor.matmul(
                            ps[:], w[:, c, :], xp[j][:, cl, :],
                            start=(c == 0), stop=(c == NCH - 1),
                        )
                    if clo + cw < NCH:
                        warmfill(3)
                nc.scalar.activation(
                    qt[:, 0:512], ps[:], mybir.ActivationFunctionType.Copy,
                    bias=0.0, scale=SCALE,
                )

            def proj(wname, gname, dst_off, scale, dst, copy_eng="scalar"):
                w = w_sb[wname]
                ps = mm_ps.tile([128, 512], F32, tag="mm", name=f"p{wname}{gname}")
                for j, clo, cw in GRP[gname]:
                    for cl in range(cw):
                        c = clo + cl
                        nc.tensor.matmul(
                            ps[:], w[:, c, :], xp[j][:, cl, :],
                            start=(c == 0), stop=(c == NCH - 1),
                        )
                dslice = dst[:, dst_off:dst_off + 512]
                if scale is not None and copy_eng == "vector":
                    # keep ACT free for the exps in the endgame
                    nc.vector.tensor_scalar_mul(dslice, ps[:], scale)
                elif scale is not None:
                    nc.scalar.activation(
                        dslice, ps[:], mybir.ActivationFunctionType.Copy,
                        bias=0.0, scale=scale,
                    )
                else:
                    nc.scalar.copy(dslice, ps[:])

            def vproj(gname, kbs):
                w = w_sb["wv"]
                base = kbs[0]
                for kb in kbs:
                    vps = mm_ps.tile([128, 128], F32, tag="mm", name=f"v{kb}")
                    lo = (kb - base) * 128
                    for j, clo, cw in GRP[gname]:
                        for cl in range(cw):
                            c = clo + cl
                            nc.tensor.matmul(
                                vps[:], xp[j][:, cl, lo:lo + 128], w[:, c, :],
                                start=(c == 0), stop=(c == NCH - 1),
                            )
                    nc.vector.tensor_copy(
                        va[:, kb * 129:kb * 129 + DV], vps[:])

            def scores(half, gl, ss):
                for s in ss:
                    lo = max(s * 128, gl * 512)
                    if lo < (gl + 1) * 512:
                        a = lo - gl * 512
                        g = 2 * half + gl
                        st = mm_ps.tile([128, 512], F32, tag="mm", name=f"s{s}{g}")
                        nc.tensor.matmul(
                            st[:, a:512],
                            kt[:, s * 128:(s + 1) * 128],
                            qt[:, g * 512 + a:(g + 1) * 512],
                            start=True, stop=True,
                            skip_group_check=True,
                        )
                        if gl == s // 4:
                            qoff = (s % 4) * 128
                            nc.vector.tensor_add(
                                st[:, qoff:qoff + 128],
                                st[:, qoff:qoff + 128],
                                mask_sb[:, half * 128:(half + 1) * 128],
                            )
                        nc.scalar.activation(
                            at[g][:, s * 512 + a:(s + 1) * 512], st[:, a:512],
                            mybir.ActivationFunctionType.Exp,
                            bias=0.0, scale=1.0,
                        )

            def av(hs):
                # two q-block panels share one z psum bank (disjoint column
                # ranges, sequential start/stop groups): halves the DVE
                # copy count and the chain-to-copy slot serialization
                for h0 in list(hs)[::2]:
                    zp = z_ps.tile([128, 2 * (DV + 1)], F32, tag="z",
                                   name=f"z{h0}")
                    for i in (0, 1):
                        h = h0 + i
                        g, q, smax = h // 4, (h % 4) * 128, h % 8
                        for s in range(smax + 1):
                            nc.tensor.matmul(
                                zp[:, i * 129:(i + 1) * 129],
                                at[g][:, s * 512 + q:s * 512 + q + 128],
                                va[:, s * 129:(s + 1) * 129],
                                start=(s == 0), stop=(s == smax),
                                skip_group_check=True,
                            )
                    nc.vector.tensor_copy(
                        z_sb[:, h0 * 129:(h0 + 2) * 129], zp[:])
                    h = h0 + 1
                    if h == 13 or h == 15:
                        lo2, hi2 = (h - 1) * 129, (h + 1) * 129
                        nc.sync.dma_start(
                            out=out_ext[:, lo2:hi2], in_=z_sb[:, lo2:hi2])
                    elif h % 4 == 3:
                        g = h // 4
                        nc.sync.dma_start(
                            out=out_ext[:, g * 4 * 129:(g + 1) * 4 * 129],
                            in_=z_sb[:, g * 4 * 129:(g + 1) * 4 * 129],
                        )

            proj_q_A()
            proj("wk", "A", 0, None, kt)
            vproj("A", [0, 1, 2, 3])
            scores(0, 0, range(0, 4))
            proj("wq", "B", 512, SCALE, qt)
            proj("wk", "B", 512, None, kt)
            vproj("B", [4, 5, 6, 7])
            scores(0, 1, range(0, 8))
            av(range(0, 4))
            proj("wq", "C", 1024, SCALE, qt, copy_eng="vector")
            scores(1, 0, range(0, 4))
            proj("wq", "D", 1536, SCALE, qt, copy_eng="vector")
            scores(1, 1, range(0, 4))
            av(range(4, 8))
            scores(1, 1, range(4, 8))
            av(range(8, 12))
            av(range(12, 16))

    nc.finalize()
    return nc


_NC = None


def _get_nc():
    global _NC
    if _NC is None:
        _NC = build_nc()
    return _NC


def _make_mask(par):
    r = np.arange(128)[:, None]
    q = np.arange(128)[None, :]
    triT = np.where(r <= q, 0.0, MASKVAL).astype(np.float32)
    other = (np.full((128, 128), MASKVAL, np.float32) if par
             else np.zeros((128, 128), np.float32))
    return np.ascontiguousarray(np.concatenate([triT, other], axis=1))


def kernel(X, W_Q, W_K, W_V):
    X = np.asarray(X, np.float32)
    W_Q = np.asarray(W_Q, np.float32)
    W_K = np.asarray(W_K, np.float32)
    W_V = np.asarray(W_V, np.float32)

    nc = _get_nc()

    def warr(W):
        return np.ascontiguousarray(
            W.astype(ml_dtypes.bfloat16).reshape(NCH, 128, DK)
            .transpose(1, 0, 2).reshape(128, NCH * DK)
        )

    wq = warr(W_Q)
    wkv = np.ascontiguousarray(np.concatenate([warr(W_K), warr(W_V)], axis=1))
    masks = [_make_mask(0), _make_mask(1)]

    xt_cache = {}
    in_maps = []
    for c in range(8):
        b, p = c // 2, c % 2
        if b not in xt_cache:
            xt_cache[b] = np.ascontiguousarray(X[b].T).astype(ml_dtypes.bfloat16)
        xt = xt_cache[b]
        own = [2 * k + p for k in range(SLOTS)]
        oth = [2 * k + 1 - p for k in range(SLOTS)]
        colidx = np.concatenate(
            [np.arange(m * 128, (m + 1) * 128) for m in own + oth])
        parts = []
        for lo, w, clo, cw in PIECES:
            sub = xt[:, colidx[lo:lo + w]]
            sub = sub.reshape(NCH, 128, w)[clo:clo + cw]
            parts.append(sub.transpose(1, 0, 2).reshape(128, cw * w))
        xin = np.ascontiguousarray(np.concatenate(parts, axis=1))
        head = np.ascontiguousarray(
            np.concatenate([wq, xin[:, 0:2 * 512]], axis=1))
        in_maps.append({
            "xin": xin, "head": head, "wkv": wkv, "maskT": masks[p],
        })

    res = run_bass_kernel_spmd(nc, in_maps, list(range(8)))

    NUM = np.zeros((B, L, DV), np.float32)
    DEN = np.zeros((B, L, 1), np.float32)
    for c in range(8):
        b, p = c // 2, c % 2
        o = np.asarray(res.results[c]["out"], np.float32)
        for h in range(QB):
            j = 2 * (h % 8) + (p if h < 8 else 1 - p)
            blk = o[:, h * 129:(h + 1) * 129]
            NUM[b, j * 128:(j + 1) * 128, :] += blk[:, :DV]
            DEN[b, j * 128:(j + 1) * 128, 0] += blk[:, DV]
    return NUM / DEN


# revision 26
# speedup vs baseline: 1.2396x; 1.2396x over previous
"""Causal attention (B=4, L=2048, d_model=1024, d_k=d_v=128) on 8 TRN2 NeuronCores.

Key-parity split; v5 snapshot (46430 ns): serialized sync input DMAs,
merged tiles, V projected directly in [key, v] layout, scalar-engine
proj copies, additive f32 causal mask pre-exp on DVE, out DMAs on scalar.
"""

import sys

sys.path.insert(0, "/opt/trn_rl_repo")
sys.path.insert(0, "/opt/trn_rl_repo/concourse")

import ml_dtypes
import numpy as np

import concourse.bass as bass  # noqa: F401
import concourse.mybir as mybir
import concourse.tile as tile
from concourse import bacc
from concourse.bass_utils import run_bass_kernel_spmd

B, L, DM, DK, DV = 4, 2048, 1024, 128, 128
NCH = DM // 128
SLOTS = 8
QB = 16
SCALE = float(DK) ** -0.5
MASKVAL = -1e9

F32 = mybir.dt.float32
BF16 = mybir.dt.bfloat16

PIECES = [
    (0,    512, 0, 2),
    (0,    512, 2, 2),
    (0,    512, 4, 4),
    (512,  512, 0, 8),
    (1024, 512, 0, 8),
    (1536, 512, 0, 8),
]
XIN_COLS = sum(w * cw for _, w, _, cw in PIECES)  # 16384


def build_nc():
    nc = bacc.Bacc()

    xin_ext = nc.declare_dram_parameter("xin", [128, XIN_COLS], BF16, isOutput=False)
    # wq (8 chunks x 128) followed by x piece 0 (2 chunks x 512) per partition
    head_ext = nc.declare_dram_parameter("head", [128, DM + 2 * 512], BF16,
                                         isOutput=False)
    wkv_ext = nc.declare_dram_parameter("wkv", [128, 2 * DM], BF16, isOutput=False)
    mask_ext = nc.declare_dram_parameter("maskT", [128, 256], F32, isOutput=False)
    out_ext = nc.declare_dram_parameter("out", [128, QB * 129], F32, isOutput=True)

    with tile.TileContext(nc) as tc:
        with (
            tc.tile_pool(name="persist", bufs=1) as persist,
            tc.tile_pool(name="mm_ps", bufs=6, space="PSUM") as mm_ps,
            tc.tile_pool(name="z_ps", bufs=2, space="PSUM") as z_ps,
        ):
            head_sb = persist.tile([128, DM + 2 * 512], BF16, tag="head",
                                   name="head")
            nc.sync.dma_start(out=head_sb[:], in_=head_ext[:])
            wq_sb = head_sb[:, 0:DM].rearrange("p (c d) -> p c d", d=128)

            xp = [head_sb[:, DM:DM + 2 * 512].rearrange("p (c w) -> p c w", w=512)]

            def load_piece(j):
                _, w, _, cw = PIECES[j]
                off = sum(wi * cwi for _, wi, _, cwi in PIECES[:j])
                t = persist.tile([128, cw, w], BF16, tag=f"xp{j}", name=f"xp{j}")
                nc.sync.dma_start(
                    out=t[:],
                    in_=xin_ext[:, off:off + cw * w].rearrange(
                        "p (c w) -> p c w", w=w
                    ),
                )
                return t

            wkv_sb = persist.tile([128, 2, NCH, 128], BF16, tag="wkv", name="wkv")
            nc.sync.dma_start(
                out=wkv_sb[:],
                in_=wkv_ext.rearrange("p (i c d) -> p i c d", i=2, d=128))
            xp.append(load_piece(1))
            xp.append(load_piece(2))
            mask_sb = persist.tile([128, 256], F32, tag="mask")
            nc.scalar.dma_start(out=mask_sb[:], in_=mask_ext[:])
            for j in range(3, len(PIECES)):
                xp.append(load_piece(j))

            w_sb = {"wq": wq_sb, "wk": wkv_sb[:, 0], "wv": wkv_sb[:, 1]}

            qt = persist.tile([128, 4 * 512], BF16, tag="qt", name="qt")
            kt = persist.tile([128, 2 * 512], BF16, tag="kt", name="kt")
            va = persist.tile([128, SLOTS * (DV + 1)], BF16, tag="va", name="va")
            nc.vector.memset(va[:], 1.0)
            at = [persist.tile([128, SLOTS * 512], BF16, tag=f"atg{g}",
                               name=f"atg{g}") for g in range(4)]
            z_sb = persist.tile([128, QB * 129], F32, tag="zsb", name="zsb")

            GRP = {"A": [(0, 0, 2), (1, 2, 2), (2, 4, 4)], "B": [(3, 0, 8)],
                   "C": [(4, 0, 8)], "D": [(5, 0, 8)]}

            def warmfill(n):
                # junk matmuls, result never read: keep the PE clock hot
                # while DMA fills (PE is in-order; these slot into stalls)
                wt = mm_ps.tile([128, 256], F32, tag="mm", name="warm")
                for wi in range(n):
                    nc.tensor.matmul(
                        wt[:], va[:, 0:128], va[:, 0:256],
                        start=(wi == 0), stop=(wi == n - 1),
                    )

            warmfill(13)

            def proj_q_A():
                # Q over column group A with warm fillers at the piece
                # boundaries where the DMA stream is still catching up
                w = w_sb["wq"]
                ps = mm_ps.tile([128, 512], F32, tag="mm", name="pwqA")
                for j, clo, cw in GRP["A"]:
                    for cl in range(cw):
                        c = clo + cl
                        nc.tensor.matmul(
                            ps[:], w[:, c, :], xp[j][:, cl, :],
                            start=(c == 0), stop=(c == NCH - 1),
                        )
                    if clo + cw < NCH:
                        warmfill(3)
                nc.scalar.activation(
                    qt[:, 0:512], ps[:], mybir.ActivationFunctionType.Copy,
                    bias=0.0, scale=SCALE,
                )

            def proj(wname, gname, dst_off, scale, dst, copy_eng="scalar"):
                w = w_sb[wname]
                ps = mm_ps.tile([128, 512], F32, tag="mm", name=f"p{wname}{gname}")
                for j, clo, cw in GRP[gname]:
                    for cl in range(cw):
                        c = clo + cl
                        nc.tensor.matmul(
                            ps[:], w[:, c, :], xp[j][:, cl, :],
                            start=(c == 0), stop=(c == NCH - 1),
                        )
                dslice = dst[:, dst_off:dst_off + 512]
                if scale is not None and copy_eng == "vector":
                    # keep ACT free for the exps in the endgame
                    nc.vector.tensor_scalar_mul(dslice, ps[:], scale)
                elif scale is not None:
                    nc.scalar.activation(
                        dslice, ps[:], mybir.ActivationFunctionType.Copy,
                        bias=0.0, scale=scale,
                    )
                else:
                    nc.scalar.copy(dslice, ps[:])

            def vproj(gname, kbs):
                w = w_sb["wv"]
                base = kbs[0]
                for kb in kbs:
                    vps = mm_ps.tile([128, 128], F32, tag="mm", name=f"v{kb}")
                    lo = (kb - base) * 128
                    for j, clo, cw in GRP[gname]:
                        for cl in range(cw):
                            c = clo + cl
                            nc.tensor.matmul(
                                vps[:], xp[j][:, cl, lo:lo + 128], w[:, c, :],
                                start=(c == 0), stop=(c == NCH - 1),
                            )
                    nc.vector.tensor_copy(
                        va[:, kb * 129:kb * 129 + DV], vps[:])

            def scores(half, gl, ss):
                for s in ss:
                    lo = max(s * 128, gl * 512)
                    if lo < (gl + 1) * 512:
                        a = lo - gl * 512
                        g = 2 * half + gl
                        st = mm_ps.tile([128, 512], F32, tag="mm", name=f"s{s}{g}")
                        nc.tensor.matmul(
                            st[:, a:512],
                            kt[:, s * 128:(s + 1) * 128],
                            qt[:, g * 512 + a:(g + 1) * 512],
                            start=True, stop=True,
                            skip_group_check=True,
                        )
                        if gl == s // 4:
                            qoff = (s % 4) * 128
                            nc.vector.tensor_add(
                                st[:, qoff:qoff + 128],
                                st[:, qoff:qoff + 128],
                                mask_sb[:, half * 128:(half + 1) * 128],
                            )
                        nc.scalar.activation(
                            at[g][:, s * 512 + a:(s + 1) * 512], st[:, a:512],
                            mybir.ActivationFunctionType.Exp,
                            bias=0.0, scale=1.0,
                        )

            def av(hs):
                # two q-block panels share one z psum bank (disjoint column
                # ranges, sequential start/stop groups): halves the DVE
                # copy count and the chain-to-copy slot serialization
                for h0 in list(hs)[::2]:
                    zp = z_ps.tile([128, 2 * (DV + 1)], F32, tag="z",
                                   name=f"z{h0}")
                    for i in (0, 1):
                        h = h0 + i
                        g, q, smax = h // 4, (h % 4) * 128, h % 8
                        for s in range(smax + 1):
                            nc.tensor.matmul(
                                zp[:, i * 129:(i + 1) * 129],
                                at[g][:, s * 512 + q:s * 512 + q + 128],
                                va[:, s * 129:(s + 1) * 129],
                                start=(s == 0), stop=(s == smax),
                                skip_group_check=True,
                            )
                    nc.vector.tensor_copy(
                        z_sb[:, h0 * 129:(h0 + 2) * 129], zp[:])
                    h = h0 + 1
                    if h == 13 or h == 15:
                        lo2, hi2 = (h - 1) * 129, (h + 1) * 129
                        nc.sync.dma_start(
                            out=out_ext[:, lo2:hi2], in_=z_sb[:, lo2:hi2])
                    elif h % 4 == 3:
                        g = h // 4
                        nc.sync.dma_start(
                            out=out_ext[:, g * 4 * 129:(g + 1) * 4 * 129],
                            in_=z_sb[:, g * 4 * 129:(g + 1) * 4 * 129],
                        )

            proj_q_A()
            proj("wk", "A", 0, None, kt)
            vproj("A", [0, 1, 2, 3])
            scores(0, 0, range(0, 4))
            proj("wq", "B", 512, SCALE, qt)
            proj("wk", "B", 512, None, kt)
            vproj("B", [4, 5, 6, 7])
            scores(0, 1, range(0, 8))
            av(range(0, 4))
            proj("wq", "C", 1024, SCALE, qt, copy_eng="vector")
            scores(1, 0, range(0, 4))
            proj("wq", "D", 1536, SCALE, qt, copy_eng="vector")
            scores(1, 1, range(0, 4))
            av(range(4, 8))
            scores(1, 1, range(4, 8))
            av(range(8, 12))
            av(range(12, 16))

    nc.finalize()
    return nc


_NC = None


def _get_nc():
    global _NC
    if _NC is None:
        _NC = build_nc()
    return _NC


def _make_mask(par):
    r = np.arange(128)[:, None]
    q = np.arange(128)[None, :]
    triT = np.where(r <= q, 0.0, MASKVAL).astype(np.float32)
    other = (np.full((128, 128), MASKVAL, np.float32) if par
             else np.zeros((128, 128), np.float32))
    return np.ascontiguousarray(np.concatenate([triT, other], axis=1))


def kernel(X, W_Q, W_K, W_V):
    X = np.asarray(X, np.float32)
    W_Q = np.asarray(W_Q, np.float32)
    W_K = np.asarray(W_K, np.float32)
    W_V = np.asarray(W_V, np.float32)

    nc = _get_nc()

    def warr(W):
        return np.ascontiguousarray(
            W.astype(ml_dtypes.bfloat16).reshape(NCH, 128, DK)
            .transpose(1, 0, 2).reshape(128, NCH * DK)
        )

    wq = warr(W_Q)
    wkv = np.ascontiguousarray(np.concatenate([warr(W_K), warr(W_V)], axis=1))
    masks = [_make_mask(0), _make_mask(1)]

    xt_cache = {}
    in_maps = []
    for c in range(8):
        b, p = c // 2, c % 2
        if b not in xt_cache:
            xt_cache[b] = np.ascontiguousarray(X[b].T).astype(ml_dtypes.bfloat16)
        xt = xt_cache[b]
        own = [2 * k + p for k in range(SLOTS)]
        oth = [2 * k + 1 - p for k in range(SLOTS)]
        colidx = np.concatenate(
            [np.arange(m * 128, (m + 1) * 128) for m in own + oth])
        parts = []
        for lo, w, clo, cw in PIECES:
            sub = xt[:, colidx[lo:lo + w]]
            sub = sub.reshape(NCH, 128, w)[clo:clo + cw]
            parts.append(sub.transpose(1, 0, 2).reshape(128, cw * w))
        xin = np.ascontiguousarray(np.concatenate(parts, axis=1))
        head = np.ascontiguousarray(
            np.concatenate([wq, xin[:, 0:2 * 512]], axis=1))
        in_maps.append({
            "xin": xin, "head": head, "wkv": wkv, "maskT": masks[p],
        })

    res = run_bass_kernel_spmd(nc, in_maps, list(range(8)))

    NUM = np.zeros((B, L, DV), np.float32)
    DEN = np.zeros((B, L, 1), np.float32)
    for c in range(8):
        b, p = c // 2, c % 2
        o = np.asarray(res.results[c]["out"], np.float32)
        for h in range(QB):
            j = 2 * (h % 8) + (p if h < 8 else 1 - p)
            blk = o[:, h * 129:(h + 1) * 129]
            NUM[b, j * 128:(j + 1) * 128, :] += blk[:, :DV]
            DEN[b, j * 128:(j + 1) * 128, 0] += blk[:, DV]
    return NUM / DEN
